# revision 1
# baseline (speedup 1.0000x reference)
"""BitLinear (out = input @ sign(weight).T + bias) on 8 Trainium2 NeuronCores.

Full shapes: input [4, 2048, 4096] f32, weight [4096, 4096] f32, bias [4096] f32.
Sharding: tokens (4*2048=8192) 4-way x out-features 2-way -> 8 cores, each
computing out[2048 tok, 2048 out] = x_shard @ sign(W_shard).T + bias_shard.

Device kernel per core: weight streamed in o-major 512-wide stripes,
sign-cast to fp16 (exact +-1) into a resident SBUF tensor; x token-tiles
DMA-cast f32->fp16 (SWDGE); fp16 matmuls accumulate over K=4096 in fp32
PSUM; bias added during PSUM->SBUF copy; f32 result DMAd out.
"""

from contextlib import ExitStack

import numpy as np

TOK_FULL, OUT_FULL, K_FULL = 8192, 4096, 4096
TG, OG = 4, 2              # token groups x out-feature groups = 8 cores
T = TOK_FULL // TG         # 2048 tokens per core
O = OUT_FULL // OG         # 2048 out features per core
P = 128
OB = 512

_CACHE = {}


def _build_nc():
    import concourse.tile as tile
    from concourse import bacc, mybir

    F32 = mybir.dt.float32
    F16 = mybir.dt.float16
    NT, NKO, NOB = T // P, K_FULL // P, O // OB

    nc = bacc.Bacc("TRN2", target_bir_lowering=False, debug=False,
                   num_devices=8)
    xt = nc.declare_dram_parameter("xt", [NT, P, NKO, P], F32, isOutput=False)
    wt = nc.declare_dram_parameter("wt", [NKO, P, O], F32, isOutput=False)
    bias = nc.declare_dram_parameter("bias", [O], F32, isOutput=False)
    out = nc.declare_dram_parameter("out", [T, O], F32, isOutput=True)

    with tile.TileContext(nc) as tc, ExitStack() as ctx:
        s_pool = ctx.enter_context(tc.tile_pool(name="s", bufs=1))
        w_pool = ctx.enter_context(tc.tile_pool(name="w", bufs=4))
        x_pool = ctx.enter_context(tc.tile_pool(name="x", bufs=2))
        o_pool = ctx.enter_context(tc.tile_pool(name="o", bufs=3))
        ps_pool = ctx.enter_context(tc.tile_pool(name="ps", bufs=4, space="PSUM"))

        S = s_pool.tile([P, NKO, O], F16)          # resident sign(W) fp16
        bias_sb = s_pool.tile([P, O], F32)
        nc.sync.dma_start(bias_sb[:], bias.ap().partition_broadcast(P))

        # o-major stripes so matmuls on ob=0 start after 1/NOB of W arrived
        for ob in range(NOB):
            osl = slice(ob * OB, (ob + 1) * OB)
            for ko in range(NKO):
                wst = w_pool.tile([P, OB], F32)
                nc.sync.dma_start(wst[:], wt[ko, :, osl])
                nc.scalar.sign(S[:, ko, osl], wst[:])

        for t in range(NT):
            xf = x_pool.tile([P, NKO, P], F16)
            nc.gpsimd.dma_start(xf[:], xt[t])      # SWDGE f32->fp16 cast
            tsl = slice(t * P, (t + 1) * P)
            for ob in range(NOB):
                osl = slice(ob * OB, (ob + 1) * OB)
                ps = ps_pool.tile([P, OB], F32)
                for ko in range(NKO):
                    nc.tensor.matmul(
                        ps[:], lhsT=xf[:, ko, :], rhs=S[:, ko, osl],
                        start=(ko == 0), stop=(ko == NKO - 1),
                    )
                ost = o_pool.tile([P, OB], F32)
                nc.vector.tensor_add(out=ost[:], in0=ps[:], in1=bias_sb[:, osl])
                nc.sync.dma_start(out[tsl, osl], ost[:])

    nc.compile()
    return nc


def _get_exec():
    """Build (once) the jitted 8-core executable. Returns (fn, n_cores)."""
    if "exec" in _CACHE:
        return _CACHE["exec"]

    import jax
    import jax.numpy as jnp
    from jax.sharding import Mesh, PartitionSpec
    from jax.experimental.shard_map import shard_map
    from concourse import bass2jax, mybir

    nc = _build_nc()
    bass2jax.install_neuronx_cc_hook()
    partition_name = (nc.partition_id_tensor.name
                      if nc.partition_id_tensor else None)

    in_names, out_names, out_avals = [], [], []
    for alloc in nc.m.functions[0].allocations:
        if not isinstance(alloc, mybir.MemoryLocationSet):
            continue
        name = alloc.memorylocations[0].name
        if alloc.kind == "ExternalInput":
            if name != partition_name:
                in_names.append(name)
        elif alloc.kind == "ExternalOutput":
            out_names.append(name)
            out_avals.append(jax.core.ShapedArray(
                tuple(alloc.tensor_shape), mybir.dt.np(alloc.dtype)))
    n_params = len(in_names)
    all_names = tuple(in_names + out_names)
    if partition_name is not None:
        all_names = all_names + (partition_name,)

    def _body(*args):
        extra = ((bass2jax.partition_id_tensor(),)
                 if partition_name is not None else ())
        outs = bass2jax._bass_exec_p.bind(
            *args, *extra,
            out_avals=tuple(out_avals),
            in_names=all_names,
            out_names=tuple(out_names),
            lowering_input_output_aliases=(),
            sim_require_finite=True,
            sim_require_nnan=True,
            nc=nc,
        )
        return tuple(outs)

    devices = jax.devices()[:8]
    mesh = Mesh(np.asarray(devices), ("core",))
    sharded = jax.jit(shard_map(
        _body, mesh=mesh,
        in_specs=(PartitionSpec("core"),) * (n_params + len(out_names)),
        out_specs=(PartitionSpec("core"),) * len(out_names),
        check_rep=False,
    ))
    zero_outs = [np.zeros((8 * a.shape[0], *a.shape[1:]), a.dtype)
                 for a in out_avals]
    _CACHE["exec"] = (sharded, in_names, out_names, mesh, zero_outs)
    return _CACHE["exec"]


def _shard_inputs(input, weight, bias):
    """Pure-permutation host sharding -> concatenated global arrays."""
    NT, NKO = T // P, K_FULL // P
    x = np.ascontiguousarray(np.asarray(input, dtype=np.float32)).reshape(
        TOK_FULL, K_FULL)
    w = np.asarray(weight, dtype=np.float32)
    b = np.asarray(bias, dtype=np.float32)
    xts, wts, bs = [], [], []
    for c in range(8):
        ti, oj = c % TG, c // TG
        xs = x[ti * T:(ti + 1) * T]
        xts.append(np.ascontiguousarray(
            xs.reshape(NT, P, NKO, P).transpose(0, 3, 2, 1)))
        wts.append(np.ascontiguousarray(w[oj * O:(oj + 1) * O].T).reshape(
            NKO, P, O))
        bs.append(np.ascontiguousarray(b[oj * O:(oj + 1) * O]))
    return (np.concatenate(xts, axis=0),
            np.concatenate(wts, axis=0),
            np.concatenate(bs, axis=0))


def _unshard_output(out_global, batch_shape):
    """out_global [8*T, O] -> full [4, 2048, 4096]."""
    full = np.empty((TOK_FULL, OUT_FULL), dtype=np.float32)
    per = np.asarray(out_global).reshape(8, T, O)
    for c in range(8):
        ti, oj = c % TG, c // TG
        full[ti * T:(ti + 1) * T, oj * O:(oj + 1) * O] = per[c]
    return full.reshape(*batch_shape, OUT_FULL)


def kernel(input, weight, bias):
    input = np.asarray(input)
    batch_shape = input.shape[:-1]
    fn, in_names, out_names, mesh, zero_outs = _get_exec()
    arrs = dict(zip(["xt", "wt", "bias"], _shard_inputs(input, weight, bias)))
    outs = fn(*[arrs[n] for n in in_names], *zero_outs)
    return _unshard_output(outs[out_names.index("out")], batch_shape)

